# revision 8
# baseline (speedup 1.0000x reference)
"""Chamfer loss kernel for 8x Trainium2 NeuronCores.

Problem: pred [4, 8192, 32] f32, target [4, 8192, 32] f32 ->
scalar = mean_n min_m ||p_n - t_m|| + mean_m min_n ||p_n - t_m||
(per batch, averaged over batch and points).

Sharding: batch b (4) x row-half h (2) -> 8 cores. Core c = 2*b + h
handles pred rows [h*4096, (h+1)*4096) of batch b against the full
target of batch b.

Device kernel (per core): an augmented K=34 fp16 matmul produces the
squared-distance tile d2[n, m] in PSUM (fp32 accum). Loop: m-span jj
outer (4 x 2048), row tile i inner (32 x 128 rows). Weights (pred
tile) alternate between PE array row-halves 0:34 / 64:98 so
LDWEIGHTS for tile i+1 pulls ahead of tile i's in-flight matmuls.

PSUM evacuation is split three ways to balance ACT, DVE and DMA:
 - C tiles: Scalar engine casts PSUM -> fp16 SBUF; DVE runs one
   pair-min tree level (row direction, host finishes from the DMA'd
   candidates) and the elementwise column-min accumulate.
 - ET tiles: DVE tensor_tensor_reduce evacuates PSUM directly,
   fusing the exact row-min (staged to accl) with the fp16 cast;
   the fp16 tile is exported whole and the host covers the column
   direction for it.
 - EA tiles: Scalar casts, the fp16 tile is exported whole on the
   otherwise-idle GPSIMD SWDGE queue, and the host covers both
   reduction directions. This trades idle DMA/host capacity for DVE
   and ACT cycles.
"""

import sys

sys.path.insert(0, "/opt/trn_rl_repo")

import numpy as np

B, N, M, D = 4, 8192, 8192, 32
N_LOC = N // 2          # rows per core
K_AUG = D + 2           # 34
NI = N_LOC // 128       # 32 row tiles
SPAN = 2048             # m-elements per span (4 PSUM banks)
NJJ = M // SPAN         # 4 column spans

# Per-span schedule (same for every span):
#  ET: DVE tensor_tensor_reduce evacuation (row fused) + full export
#  EA: ACT cast + full export (host does both directions)
#  C:  ACT cast + DVE tree (row candidates) + DVE column accumulate
ET_SET = (2, 7, 13, 18, 24, 29)
EA_SET = (4, 9, 15, 20, 26, 31)
BIG = 60000.0           # fp16-safe "+inf" (d2 <= ~220 here)

TTR_SLOTS = {}
for _jj in range(NJJ):
    for _i in ET_SET:
        TTR_SLOTS[(_jj, _i)] = len(TTR_SLOTS)
N_SLOTS = len(TTR_SLOTS)  # 24

_compiled = None


def _build():
    import concourse.bacc as bacc
    import concourse.mybir as mybir
    import concourse.tile as tile

    nc = bacc.Bacc("TRN2", target_bir_lowering=False, debug=False, num_devices=8)
    f32 = mybir.dt.float32
    f16 = mybir.dt.bfloat16
    OP = mybir.AluOpType

    pt_d = nc.dram_tensor("pt", [K_AUG, N_LOC], f16, kind="ExternalInput")
    tt_d = nc.dram_tensor("tt", [K_AUG, M], f16, kind="ExternalInput")
    # rowcand[p, jj, i, q]: row-min candidates of row 128*i+p over m-span jj
    row_d = nc.dram_tensor(
        "rowcand", [128, NJJ, NI, 1024], f16, kind="ExternalOutput"
    )
    # full fp16 d2 exports for EA/ET tiles (host reduces both directions)
    sbex_d = nc.dram_tensor(
        "sbex", [128, NJJ, NI, SPAN], f16, kind="ExternalOutput"
    )
    cold_d = nc.dram_tensor("colmin_dve", [128, NJJ, SPAN], f16, kind="ExternalOutput")

    with tile.TileContext(nc) as tc:
        with (
            tc.tile_pool(name="const", bufs=1) as const,
            tc.tile_pool(name="psum", bufs=2, space="PSUM") as psum_pool,
            tc.tile_pool(name="sbbf", bufs=6) as sbbf_pool,
            tc.tile_pool(name="tree", bufs=4) as tree_pool,
        ):
            # persistent tiles
            cold = [const.tile([128, SPAN], f16, name=f"cold{j}") for j in range(NJJ)]
            for j in range(NJJ):
                nc.vector.memset(cold[j][:], BIG)

            # chunked input loads on separate tiles so the first matmuls
            # only wait for their own chunk; first-needed chunks go first.
            # operands duplicated at partition offset 64 so LDWEIGHTS for
            # row tile i+1 (other PE row-half) pulls ahead of tile i's MMs.
            ptsb_c = []
            ttsb_c = []
            for k in range(4):
                pchunk = const.tile([64 + K_AUG, N_LOC // 4], f16, name=f"ptc{k}")
                tchunk = const.tile([64 + K_AUG, M // 4], f16, name=f"ttc{k}")
                ptsb_c.append(pchunk)
                ttsb_c.append(tchunk)
            NL4, M4 = N_LOC // 4, M // 4
            # first-needed halves first, split across queues
            nc.sync.dma_start(
                out=ttsb_c[0][0:K_AUG, : M4 // 2], in_=tt_d.ap()[:, : M4 // 2]
            )
            nc.scalar.dma_start(
                out=ttsb_c[0][0:K_AUG, M4 // 2 :], in_=tt_d.ap()[:, M4 // 2 : M4]
            )
            nc.sync.dma_start(
                out=ttsb_c[0][64 : 64 + K_AUG, :], in_=tt_d.ap()[:, 0:M4]
            )
            nc.sync.dma_start(out=ptsb_c[0][0:K_AUG, :], in_=pt_d.ap()[:, 0:NL4])
            nc.gpsimd.dma_start(
                out=ptsb_c[0][64 : 64 + K_AUG, :], in_=pt_d.ap()[:, 0:NL4]
            )
            for k in range(1, 4):
                nc.gpsimd.dma_start(
                    out=ptsb_c[k][0:K_AUG, :], in_=pt_d.ap()[:, k * NL4 : (k + 1) * NL4]
                )
                nc.gpsimd.dma_start(
                    out=ptsb_c[k][64 : 64 + K_AUG, :],
                    in_=pt_d.ap()[:, k * NL4 : (k + 1) * NL4],
                )
                nc.sync.dma_start(
                    out=ttsb_c[k][0:K_AUG, :], in_=tt_d.ap()[:, k * M4 : (k + 1) * M4]
                )
                nc.sync.dma_start(
                    out=ttsb_c[k][64 : 64 + K_AUG, :],
                    in_=tt_d.ap()[:, k * M4 : (k + 1) * M4],
                )

            for jj in range(NJJ):
                for i in range(NI):
                    pc = ptsb_c[i // 8]
                    c0 = (i % 8) * 128
                    off = 0 if i % 2 == 0 else 64
                    ps = psum_pool.tile([128, SPAN], f32)
                    for h in range(SPAN // 512):
                        nc.tensor.matmul(
                            ps[:, h * 512 : (h + 1) * 512],
                            pc[off : off + K_AUG, c0 : c0 + 128],
                            ttsb_c[jj][off : off + K_AUG, h * 512 : (h + 1) * 512],
                            start=True,
                            stop=True,
                        )
                    sb = sbbf_pool.tile([128, SPAN], f16)
                    if i in ET_SET:
                        # DVE evacuation + full export; host does both
                        # reduction directions for this tile
                        nc.vector.tensor_copy(sb[:], ps[:])
                        nc.gpsimd.dma_start(
                            out=sbex_d.ap()[:, jj : jj + 1, i : i + 1, :], in_=sb[:]
                        )
                    elif i in EA_SET:
                        # cast + full export; host reduces both directions
                        nc.scalar.copy(sb[:], ps[:])
                        nc.gpsimd.dma_start(
                            out=sbex_d.ap()[:, jj : jj + 1, i : i + 1, :], in_=sb[:]
                        )
                    else:
                        nc.scalar.copy(sb[:], ps[:])
                        # row direction: one fp16 pair-min tree level; host
                        # finishes from the DMA'd candidates
                        u = tree_pool.tile([128, SPAN // 2], f16, tag="u")
                        nc.vector.tensor_tensor(
                            u[:], sb[:, : SPAN // 2], sb[:, SPAN // 2 :], op=OP.min
                        )
                        nc.sync.dma_start(
                            out=row_d.ap()[:, jj : jj + 1, i : i + 1, :], in_=u[:]
                        )
                        # column direction: min-accumulate on DVE
                        nc.vector.tensor_tensor(
                            cold[jj][:], sb[:], cold[jj][:], op=OP.min
                        )
                # export this span's column accumulator
                nc.sync.dma_start(
                    out=cold_d.ap()[:, jj : jj + 1, :], in_=cold[jj][:]
                )

    nc.compile()
    return nc


def _get_compiled():
    global _compiled
    if _compiled is None:
        _compiled = _build()
    return _compiled


def _make_core_inputs(pred, target):
    """Per-core augmented, transposed fp16 operands."""
    ins = []
    for c in range(8):
        b, h = c // 2, c % 2
        pl = pred[b, h * N_LOC : (h + 1) * N_LOC]  # [N_LOC, 32]
        tg = target[b]  # [M, 32]
        pt = np.empty((K_AUG, N_LOC), dtype=np.float32)
        pt[:D] = -2.0 * pl.T
        pt[D] = np.sum(pl * pl, axis=1)
        pt[D + 1] = 1.0
        tt = np.empty((K_AUG, M), dtype=np.float32)
        tt[:D] = tg.T
        tt[D] = 1.0
        tt[D + 1] = np.sum(tg * tg, axis=1)
        import ml_dtypes
        bf16 = ml_dtypes.bfloat16
        ins.append(
            {
                "pt": np.ascontiguousarray(pt.astype(bf16)),
                "tt": np.ascontiguousarray(tt.astype(bf16)),
            }
        )
    return ins


_EXP_I = sorted(set(ET_SET) | set(EA_SET))
_C_I = [i for i in range(NI) if i not in _EXP_I]


def _finish(results):
    """Host tail: combine per-core partial minima into the scalar loss."""
    row_sum = 0.0
    col_sum = 0.0
    for b in range(B):
        col_d2 = None
        for h in range(2):
            r = results[2 * b + h]
            rc = np.asarray(r["rowcand"], dtype=np.float32)   # [128, jj, i, 1024]
            ex = np.asarray(r["sbex"])                        # [128, jj, i, 2048] f16
            rm = np.full((128, NI), np.inf, dtype=np.float32)
            # C tiles: tree candidates
            rm[:, _C_I] = rc[:, :, _C_I, :].min(axis=3).min(axis=1)
            # exported tiles: host row-min
            ex_e = ex[:, :, _EXP_I, :].astype(np.float32)     # [128, jj, nexp, 2048]
            rm[:, _EXP_I] = np.minimum(rm[:, _EXP_I], ex_e.min(axis=3).min(axis=1))
            row_sum += np.sum(np.sqrt(np.maximum(rm.astype(np.float64), 0.0)))
            # columns: device accumulator (C tiles) + host over exports
            cd = np.asarray(r["colmin_dve"], dtype=np.float32)  # [128, jj, 2048]
            cm = np.minimum(cd, ex_e.min(axis=2)).min(axis=0).reshape(M)
            col_d2 = cm if col_d2 is None else np.minimum(col_d2, cm)
        col_sum += np.sum(np.sqrt(np.maximum(col_d2.astype(np.float64), 0.0)))
    total = row_sum / (B * N) + col_sum / (B * M)
    return np.array(total, dtype=np.float32)


def kernel(pred, target, **run_kwargs):
    from concourse.bass_utils import run_bass_kernel_spmd

    pred = np.asarray(pred, dtype=np.float32)
    target = np.asarray(target, dtype=np.float32)
    nc = _get_compiled()
    ins = _make_core_inputs(pred, target)
    res = run_bass_kernel_spmd(nc, ins, list(range(8)), **run_kwargs)
    out = _finish(res.results)
    if run_kwargs:
        return out, res
    return out
